# revision 4
# baseline (speedup 1.0000x reference)
"""Trainium2 Bass kernel for IntegratedTGNLayer.

Strategy (per spec sharding hint): data-parallel over the 65536 events
across 8 NeuronCores. Node-memory gather/scatter is done host-side (the
per-event gathered rows are the data-parallel input; ids are disjoint so
the scatter is a pure indexed write). The device runs the dense compute:
2x message MLP, 2x GRU cell, 2x output projection = ~842 GFLOP.

Device layout: feature-major ([feature, event]) everywhere so every
matmul contracts over the SBUF partition dim with weights stationary:
  out[o_chunk(128), ev(512)] += W_T[k_chunk(128), o_chunk(128)].T @ X[k_chunk(128), ev(512)]
All matmul operands are bf16 (PE 1 cycle/row, FWL weight loads); PSUM
accumulates fp32; biases applied during PSUM evacuation on ScalarE.
"""

import sys

sys.path.insert(0, "/opt/trn_rl_repo")

import numpy as np
import ml_dtypes

B = 65536
M = 512
E = 512
T = 128
MSG = 512
N_NODES = 200000
NCORES = 8
BC = B // NCORES          # events per core = 8192
EV = 512                  # events per tile
NEV = BC // EV            # event tiles per core = 16

BF16 = ml_dtypes.bfloat16

_EXEC_CACHE = {}
LAST_DEVICE_SECONDS = None


def _build_nc():
    import concourse.bass as bass  # noqa: F401
    import concourse.tile as tile
    from concourse import bacc, mybir

    dt = mybir.dt
    AF = mybir.ActivationFunctionType
    OP = mybir.AluOpType

    nc = bacc.Bacc("TRN2", target_bir_lowering=False, debug=False,
                   num_devices=NCORES)

    def din(name, shape, dty=dt.bfloat16):
        return nc.dram_tensor(name, shape, dty, kind="ExternalInput").ap()

    def dout(name, shape, dty=dt.bfloat16):
        return nc.dram_tensor(name, shape, dty, kind="ExternalOutput").ap()

    # activations, feature-major [feat, BC]
    smem_d = din("smem", (M, BC))
    dmem_d = din("dmem", (M, BC))
    edge_d = din("edge", (E, BC))
    temb_d = din("temb", (T, BC))
    semb_d = din("semb", (M, BC))
    demb_d = din("demb", (M, BC))
    # weights, [in, out] (already transposed on host)
    w1_d = din("w1t", (2 * M + E + T, MSG))     # [1664, 512]
    w2_d = din("w2t", (MSG, MSG))               # [512, 512]
    wih_d = din("wiht", (MSG, 3 * M))           # [512, 1536]
    whh_d = din("whht", (M, 3 * M))             # [512, 1536]
    ow_d = din("owt", (2 * M, M))               # [1024, 512]
    bias_d = din("biasp", (128, 28), dt.float32)
    # outputs, feature-major
    upds_d = dout("upds", (M, BC))
    updd_d = dout("updd", (M, BC))
    outs_d = dout("outs", (M, BC))
    outd_d = dout("outd", (M, BC))

    def part(d_ap, kc):
        # [K*128, N] dram -> [128, kc, N] (partition-chunked)
        return d_ap.rearrange("(k p) n -> p k n", p=128, k=kc)

    with tile.TileContext(nc) as tc:
        with tc.tile_pool(name="w", bufs=1) as wp, \
             tc.tile_pool(name="inp", bufs=2) as ip, \
             tc.tile_pool(name="emb", bufs=2) as ep, \
             tc.tile_pool(name="h1", bufs=2) as hp, \
             tc.tile_pool(name="msg", bufs=2) as mp, \
             tc.tile_pool(name="rz", bufs=2) as rzp, \
             tc.tile_pool(name="sm", bufs=3) as sp, \
             tc.tile_pool(name="upd", bufs=3) as up, \
             tc.tile_pool(name="out", bufs=3) as op_, \
             tc.tile_pool(name="ps", bufs=4, space="PSUM") as pp, \
             tc.tile_pool(name="ps2", bufs=2, space="PSUM") as pp2:

            # ---- resident weights ----
            w1 = wp.tile([128, 13 * 512], dt.bfloat16)
            nc.sync.dma_start(w1[:].rearrange("p (k n) -> p k n", n=512),
                              part(w1_d, 13))
            w2 = wp.tile([128, 4 * 512], dt.bfloat16)
            nc.sync.dma_start(w2[:].rearrange("p (k n) -> p k n", n=512),
                              part(w2_d, 4))
            wih = wp.tile([128, 4 * 1536], dt.bfloat16)
            nc.sync.dma_start(wih[:].rearrange("p (k n) -> p k n", n=1536),
                              part(wih_d, 4))
            whh = wp.tile([128, 4 * 1536], dt.bfloat16)
            nc.sync.dma_start(whh[:].rearrange("p (k n) -> p k n", n=1536),
                              part(whh_d, 4))
            ow = wp.tile([128, 8 * 512], dt.bfloat16)
            nc.sync.dma_start(ow[:].rearrange("p (k n) -> p k n", n=512),
                              part(ow_d, 8))
            bias = wp.tile([128, 28], dt.float32)
            nc.sync.dma_start(bias[:], bias_d[:, :])

            def w1_blk(k, o):
                return w1[:, k * 512 + o * 128: k * 512 + (o + 1) * 128]

            def w2_blk(k, o):
                return w2[:, k * 512 + o * 128: k * 512 + (o + 1) * 128]

            def wih_blk(k, o):   # o in 0..11
                return wih[:, k * 1536 + o * 128: k * 1536 + (o + 1) * 128]

            def whh_blk(k, o):
                return whh[:, k * 1536 + o * 128: k * 1536 + (o + 1) * 128]

            def ow_blk(k, o):    # k in 0..7
                return ow[:, k * 512 + o * 128: k * 512 + (o + 1) * 128]

            # bias columns: 0-3 b1 | 4-7 b2 | 8-15 bihh_rz | 16-19 b_ihn
            #               20-23 b_hhn | 24-27 out_b
            def bcol(j):
                return bias[:, j: j + 1]

            for e in range(NEV):
                ev = slice(e * EV, (e + 1) * EV)

                def load4(pool, d_ap, tag, kc=4, npart=512):
                    t = pool.tile([128, kc * npart], dt.bfloat16, tag=tag)
                    nc.sync.dma_start(
                        t[:].rearrange("p (k n) -> p k n", n=npart),
                        part(d_ap, kc)[:, :, ev])
                    return t

                xs = load4(ip, smem_d, "xs")
                xd = load4(ip, dmem_d, "xd")
                xe = load4(ip, edge_d, "xe")
                xt = ip.tile([128, EV], dt.bfloat16)
                nc.sync.dma_start(xt[:], temb_d[:, ev])

                def ch(t, k):
                    return t[:, k * EV: (k + 1) * EV]

                # ---- layer 1 (both message directions) ----
                def l1(a, b):
                    h = hp.tile([128, 4 * EV], dt.bfloat16)
                    for o in range(4):
                        ps = pp.tile([128, EV], dt.float32)
                        mmk = [(w1_blk(k, o), ch(a, k)) for k in range(4)]
                        mmk += [(w1_blk(4 + k, o), ch(b, k)) for k in range(4)]
                        mmk += [(w1_blk(8 + k, o), ch(xe, k)) for k in range(4)]
                        mmk += [(w1_blk(12, o), xt[:])]
                        for i, (lhs, rhs) in enumerate(mmk):
                            nc.tensor.matmul(ps[:], lhs, rhs,
                                             start=(i == 0), stop=(i == len(mmk) - 1))
                        nc.scalar.activation(ch(h, o), ps[:], AF.Relu,
                                             bias=bcol(o))
                    return h

                h1s = l1(xs, xd)   # message(src_mem, dst_mem, ...)
                h1d = l1(xd, xs)

                # ---- layer 2 ----
                def l2(h1):
                    m = mp.tile([128, 4 * EV], dt.bfloat16)
                    for o in range(4):
                        ps = pp.tile([128, EV], dt.float32)
                        for k in range(4):
                            nc.tensor.matmul(ps[:], w2_blk(k, o), ch(h1, k),
                                             start=(k == 0), stop=(k == 3))
                        nc.scalar.activation(ch(m, o), ps[:], AF.Identity,
                                             bias=bcol(4 + o))
                    return m

                msd = l2(h1s)      # src->dst message
                mds = l2(h1d)      # dst->src message

                # ---- GRU cell: upd = (1-z)*n + z*h ----
                def gru(x, h):
                    rz = rzp.tile([128, 8 * EV], dt.bfloat16)
                    for o in range(8):
                        ps = pp.tile([128, EV], dt.float32)
                        for k in range(4):
                            nc.tensor.matmul(ps[:], wih_blk(k, o), ch(x, k),
                                             start=(k == 0), stop=False)
                        for k in range(4):
                            nc.tensor.matmul(ps[:], whh_blk(k, o), ch(h, k),
                                             start=False, stop=(k == 3))
                        nc.scalar.activation(ch(rz, o), ps[:], AF.Sigmoid,
                                             bias=bcol(8 + o))
                    upd = up.tile([128, 4 * EV], dt.bfloat16)
                    for o in range(4):
                        psi = pp2.tile([128, EV], dt.float32)
                        for k in range(4):
                            nc.tensor.matmul(psi[:], wih_blk(k, 8 + o), ch(x, k),
                                             start=(k == 0), stop=(k == 3))
                        psh = pp2.tile([128, EV], dt.float32)
                        for k in range(4):
                            nc.tensor.matmul(psh[:], whh_blk(k, 8 + o), ch(h, k),
                                             start=(k == 0), stop=(k == 3))
                        hn = sp.tile([128, EV], dt.bfloat16)
                        nc.scalar.activation(hn[:], psh[:], AF.Identity,
                                             bias=bcol(20 + o))
                        rhn = sp.tile([128, EV], dt.bfloat16)
                        nc.vector.tensor_mul(rhn[:], ch(rz, o), hn[:])
                        tin = sp.tile([128, EV], dt.float32)
                        nc.vector.scalar_tensor_tensor(
                            tin[:], psi[:], bcol(16 + o), rhn[:],
                            op0=OP.add, op1=OP.add)
                        n_ = sp.tile([128, EV], dt.bfloat16)
                        nc.scalar.activation(n_[:], tin[:], AF.Tanh)
                        d = sp.tile([128, EV], dt.bfloat16)
                        nc.vector.tensor_sub(d[:], ch(h, o), n_[:])
                        zd = sp.tile([128, EV], dt.bfloat16)
                        nc.vector.tensor_mul(zd[:], ch(rz, 4 + o), d[:])
                        nc.vector.tensor_add(ch(upd, o), n_[:], zd[:])
                    return upd

                upds = gru(mds, xs)   # upd_src = GRU(dst_to_src, src_mem)
                updd = gru(msd, xd)

                nc.sync.dma_start(part(upds_d, 4)[:, :, ev],
                                  upds[:].rearrange("p (k n) -> p k n", n=EV))
                nc.sync.dma_start(part(updd_d, 4)[:, :, ev],
                                  updd[:].rearrange("p (k n) -> p k n", n=EV))

                # ---- output projection ----
                es = load4(ep, semb_d, "es")
                ed = load4(ep, demb_d, "ed")

                def oproj(u, emb, dst):
                    o_t = op_.tile([128, 4 * EV], dt.bfloat16)
                    for o in range(4):
                        ps = pp.tile([128, EV], dt.float32)
                        for k in range(4):
                            nc.tensor.matmul(ps[:], ow_blk(k, o), ch(u, k),
                                             start=(k == 0), stop=False)
                        for k in range(4):
                            nc.tensor.matmul(ps[:], ow_blk(4 + k, o), ch(emb, k),
                                             start=False, stop=(k == 3))
                        nc.scalar.activation(ch(o_t, o), ps[:], AF.Identity,
                                             bias=bcol(24 + o))
                    nc.sync.dma_start(part(dst, 4)[:, :, ev],
                                      o_t[:].rearrange("p (k n) -> p k n", n=EV))

                oproj(upds, es, outs_d)
                oproj(updd, ed, outd_d)

    nc.compile()
    return nc


def _get_nc():
    if "nc" not in _EXEC_CACHE:
        _EXEC_CACHE["nc"] = _build_nc()
    return _EXEC_CACHE["nc"]


def _fm(x):
    # [B, F] f32 -> feature-major bf16 [F, B]
    return np.ascontiguousarray(x.T).astype(BF16)


def _prep_in_maps(src_node_embeddings, dst_node_embeddings, edge_features,
                  timestamps, memory, time_w, time_b, msg_w1, msg_w2,
                  gru_w_ih, gru_w_hh, ow, bias_pack, src_ids, dst_ids):
    smem = _fm(memory[src_ids])
    dmem = _fm(memory[dst_ids])
    edge = _fm(edge_features)
    temb = np.cos(time_w[:, None] * timestamps[None, :] + time_b[:, None]
                  ).astype(BF16)
    semb = _fm(src_node_embeddings)
    demb = _fm(dst_node_embeddings)

    wmap = {
        "w1t": np.ascontiguousarray(msg_w1.T).astype(BF16),
        "w2t": np.ascontiguousarray(msg_w2.T).astype(BF16),
        "wiht": np.ascontiguousarray(gru_w_ih.T).astype(BF16),
        "whht": np.ascontiguousarray(gru_w_hh.T).astype(BF16),
        "owt": np.ascontiguousarray(ow.T).astype(BF16),
        "biasp": bias_pack,
    }
    in_maps = []
    for c in range(NCORES):
        s = slice(c * BC, (c + 1) * BC)
        in_maps.append({
            "smem": np.ascontiguousarray(smem[:, s]),
            "dmem": np.ascontiguousarray(dmem[:, s]),
            "edge": np.ascontiguousarray(edge[:, s]),
            "temb": np.ascontiguousarray(temb[:, s]),
            "semb": np.ascontiguousarray(semb[:, s]),
            "demb": np.ascontiguousarray(demb[:, s]),
            **wmap,
        })
    return in_maps


def kernel(src_node_embeddings, dst_node_embeddings, edge_features, timestamps,
           memory, time_w, time_b, msg_w1, msg_b1, msg_w2, msg_b2,
           gru_w_ih, gru_w_hh, gru_b_ih, gru_b_hh, out_w, out_b,
           src_node_ids, dst_node_ids):
    global LAST_DEVICE_SECONDS
    import time as _time
    from concourse import bass2jax

    asnp = lambda x: np.asarray(x)
    memory = asnp(memory).astype(np.float32)
    src_ids = asnp(src_node_ids)
    dst_ids = asnp(dst_node_ids)

    bihh = asnp(gru_b_ih) + asnp(gru_b_hh)
    bias_pack = np.zeros((128, 28), np.float32)
    packs = [(asnp(msg_b1), 0, 4), (asnp(msg_b2), 4, 4),
             (bihh[:1024], 8, 8), (asnp(gru_b_ih)[1024:], 16, 4),
             (asnp(gru_b_hh)[1024:], 20, 4), (asnp(out_b), 24, 4)]
    for vec, j0, nch in packs:
        for o in range(nch):
            bias_pack[:, j0 + o] = vec[o * 128:(o + 1) * 128]

    in_maps = _prep_in_maps(
        asnp(src_node_embeddings).astype(np.float32),
        asnp(dst_node_embeddings).astype(np.float32),
        asnp(edge_features).astype(np.float32),
        asnp(timestamps).astype(np.float32),
        memory, asnp(time_w).astype(np.float32), asnp(time_b).astype(np.float32),
        asnp(msg_w1).astype(np.float32), asnp(msg_w2).astype(np.float32),
        asnp(gru_w_ih).astype(np.float32), asnp(gru_w_hh).astype(np.float32),
        asnp(out_w).astype(np.float32), bias_pack, src_ids, dst_ids)

    nc = _get_nc()
    _EXEC_CACHE["last_in_maps"] = in_maps
    t0 = _time.time()
    res = bass2jax.run_bass_via_pjrt(nc, in_maps, n_cores=NCORES)
    LAST_DEVICE_SECONDS = _time.time() - t0

    def gather(name):
        fm = np.concatenate([res[c][name].astype(np.float32)
                             for c in range(NCORES)], axis=1)
        return np.ascontiguousarray(fm.T)          # [B, 512]

    upd_src = gather("upds")
    upd_dst = gather("updd")
    src_out = gather("outs")
    dst_out = gather("outd")

    output = np.concatenate([src_out, dst_out], axis=0)
    new_memory = memory.copy()
    new_memory[src_ids] = upd_src
    new_memory[dst_ids] = upd_dst
    return output, new_memory
